# revision 41
# baseline (speedup 1.0000x reference)
"""Multi-head attention (B=8, N=1024, C=768, H=12) on 8 Trainium2 NeuronCores.

Sharding: data-parallel, one batch element per core. Each core computes the
full attention block for its batch: QKV projection, per-head softmax(QK^T/8)V,
and the output projection, entirely on-chip (SBUF/PSUM).

Layout strategy (chosen so no on-device transposes are needed):
  - host passes x^T [C, N], w_qkv^T [C, 3C], w_proj^T [C, C], bias replicated
    to [128, C].
  - Q, K are produced transposed ([d, n], head-dim on partitions) by the QKV
    matmul; V is produced in natural [n, d] layout by swapping lhsT/rhs.
  - scores are computed transposed (S^T[m, n] = K Q^T) so that exp(S^T) can be
    consumed directly as the moving operand of the P@V matmul.
  - V tiles carry an appended ones-column, so the P@V matmul's 65th output row
    is the softmax denominator (row-sum of exp scores) for free.
  - normalization multiplies by a reciprocal row broadcast across partitions
    via a DRAM-bounced DMA (SBUF APs cannot partition-broadcast).

Matmul operands use dtype float32r: single-pass PE streaming (1 column/cycle,
4x faster than float32's two-pass LOW/HIGH emulation) with 11 explicit
mantissa bits. Producers round on write; DRAM inputs are pre-rounded on host.

Scheduling: attention for head pair t overlaps the remaining QKV projection
work. All PSUM users run on half-size (single-bank) accumulation groups so
the 8 banks split 2+2 (QK/V projection) + 2+2 (scores / P@V); attn_out^T
tiles reuse the SBUF slots of dead Q^T tiles so everything fits in 192KB.
"""

import sys

import numpy as np

if "/opt/trn_rl_repo" not in sys.path:
    sys.path.insert(0, "/opt/trn_rl_repo")

B = 8
N = 1024
C = 768
H = 12
D = 64
SCALE = D ** -0.5
KT = C // 128           # 6 contraction tiles over channels
MT_QK = 2 * C // 128    # 12 output tiles for Q and K (o in [0, 1536))
NT = N // 128           # 8 token tiles
PAIRS = H // 2          # 6 head pairs

_CACHE = {}


def build_program(fast=True):
    import concourse.bacc as bacc
    import concourse.mybir as mybir
    import concourse.tile as tile

    f32 = mybir.dt.float32
    f32r = mybir.dt.float32r
    Exp = mybir.ActivationFunctionType.Exp
    fm = f32r if fast else f32

    nc = bacc.Bacc("TRN2", target_bir_lowering=False, debug=False)

    xT_d = nc.dram_tensor("xT", [C, N], fm, kind="ExternalInput")
    wqkvT_d = nc.dram_tensor("wqkvT", [C, 3 * C], fm, kind="ExternalInput")
    wprojT_d = nc.dram_tensor("wprojT", [C, C], fm, kind="ExternalInput")
    bias_d = nc.dram_tensor("bias_rep", [128, C], f32, kind="ExternalInput")
    y_d = nc.dram_tensor("y", [N, C], f32, kind="ExternalOutput")

    mm = nc.tensor.matmul

    with tile.TileContext(nc) as tc:
        # qkt/aot share one 12-slot tag: each aot[t] lands in the slot of a
        # Q^T/K^T tile that died right before it (pair t's score matmuls).
        with tc.tile_pool(name="pers", bufs=1) as pers, \
             tc.tile_pool(name="qa", bufs=13) as qa, \
             tc.tile_pool(name="cyc", bufs=2) as pB, \
             tc.tile_pool(name="dramb", bufs=2, space="DRAM") as pDr, \
             tc.tile_pool(name="ps_s", bufs=3, space="PSUM") as psS, \
             tc.tile_pool(name="ps_y", bufs=2, space="PSUM") as psY:
            # Q^T,K^T tiles [d, n]: tile m holds heads 2m (parts 0:64) and
            # 2m+1 (parts 64:128); m 0..5 = Q, 6..11 = K.
            qkt = [qa.tile([128, N], fm, name=f"qkt{m}", tag="qa")
                   for m in range(MT_QK)]
            # V tiles [n-tile, pair, 130]: per pair block [V_h0 |1| V_h1 |1];
            # ones cols at 64 and 129 feed the denominator row of P@V.
            vbuf = [pers.tile([128, PAIRS, 130], fm, name=f"vbuf{i}", tag=f"vbuf{i}")
                    for i in range(NT)]

            with tc.tile_pool(name="phA", bufs=1) as pA:
                xt = [pA.tile([128, N], fm, name=f"xt{k}", tag=f"xt{k}")
                      for k in range(KT)]
                wqk = [pA.tile([128, 2 * C], fm, name=f"wqk{k}", tag=f"wqk{k}")
                       for k in range(KT)]
                wv = [pA.tile([128, C], fm, name=f"wv{k}", tag=f"wv{k}")
                      for k in range(KT)]
                for k in range(KT):
                    nc.sync.dma_start(xt[k][:], xT_d[128 * k:128 * (k + 1), :])
                for k in range(KT):
                    nc.sync.dma_start(wv[k][:],
                                      wqkvT_d[128 * k:128 * (k + 1), 2 * C:3 * C])
                for k in range(KT):
                    nc.sync.dma_start(wqk[k][:],
                                      wqkvT_d[128 * k:128 * (k + 1), 0:2 * C])
                for i in range(NT):
                    ones_ap = vbuf[i].rearrange("p a (t c) -> p a t c", c=65)[:, :, :, 64]
                    nc.vector.memset(ones_ap.bitcast(f32), 1.0)


                # ---- QKV projection, single-bank accumulation groups ----
                def emit_qk(m):
                    for j in range(2):
                        ps = psS.tile([128, 512], f32, name="qk_ps", tag="ps")
                        for k in range(KT):
                            mm(ps[:], wqk[k][:, 128 * m:128 * (m + 1)],
                               xt[k][:, 512 * j:512 * (j + 1)],
                               start=(k == 0), stop=(k == KT - 1))
                        nc.vector.tensor_copy(qkt[m][:, 512 * j:512 * (j + 1)],
                                              ps[:])

                def emit_v(i):
                    for c0, w in ((0, 512), (512, 256)):
                        ps = psY.tile([128, 512], f32, name="v_ps", tag="py")
                        for k in range(KT):
                            mm(ps[:, 0:w], xt[k][:, 128 * i:128 * (i + 1)],
                               wv[k][:, c0:c0 + w],
                               start=(k == 0), stop=(k == KT - 1))
                        # scatter heads: even -> cols 0:64, odd -> cols 65:129
                        # within each 130-wide pair block
                        v_view = ps[:, 0:w].rearrange("p (a t c) -> p a t c",
                                                      t=2, c=64)
                        pa0 = c0 // 128
                        npair = w // 128
                        nc.vector.tensor_copy(
                            vbuf[i][:, pa0:pa0 + npair, 0:64], v_view[:, :, 0, :])
                        nc.vector.tensor_copy(
                            vbuf[i][:, pa0:pa0 + npair, 65:129], v_view[:, :, 1, :])

                # head pairs 0/1 first so attention starts while the rest
                # of the QKV projection still runs; remaining Q/K tiles are
                # emitted interleaved between attention pairs (emission order
                # drives scheduler priority).
                for i in range(NT):
                    emit_v(i)
                for m in (0, 6, 1, 7):
                    emit_qk(m)

                # remaining Q/K half-groups, injected in small chunks inside
                # the attention loops (their own psum pool keeps them off the
                # score-matmul slot chain)
                # ---- attention, j-outer so P@V psum is one bank per head ----
                # inject the remaining Q/K projection groups at pair starts,
                # spread so pairs 4/5 also get PE backfill work
                inject = {0: (2, 8), 1: (3, 9), 2: (4, 10), 3: (5,), 4: (11,)}
                for t in range(PAIRS):
                    for m in inject.get(t, ()):
                        emit_qk(m)
                    qt, kt = qkt[t], qkt[PAIRS + t]
                    aot = qa.tile([128, N], fm, name=f"aot{t}", tag="qa")
                    if t == 0:
                        aot_all = []
                    aot_all.append(aot)
                    for j in range(2):
                        pv_ps = [psY.tile([65, 512], f32, name=f"pv{h}", tag="py")
                                 for h in range(2)]
                        for i in range(NT):
                            stexp = pB.tile([128, 2, 512], fm, name="stexp",
                                            tag="stexp", bufs=4)
                            s_ps = psS.tile([128, 1024], f32, name="s_ps",
                                            tag="ps")
                            for h in range(2):
                                # S^T[m, n] = sum_d K^T[d, m] Q^T[d, n]; h0/h1
                                # use distinct PE row groups (base partition
                                # 0 / 64) and run concurrently.
                                mm(s_ps[:, 512 * h:512 * (h + 1)],
                                   kt[64 * h:64 * (h + 1), 128 * i:128 * (i + 1)],
                                   qt[64 * h:64 * (h + 1), 512 * j:512 * (j + 1)],
                                   start=True, stop=True)
                            # exp(S^T / 8) for both heads, PSUM -> SBUF f32r
                            nc.scalar.activation(
                                stexp[:, :, :],
                                s_ps[:].rearrange("p (h n) -> p h n", h=2),
                                Exp, scale=SCALE)
                            for h in range(2):
                                # rows 0:64 = (P~ @ V)^T, row 64 = denominator
                                mm(pv_ps[h][:],
                                   vbuf[i][:, t, 65 * h:65 * (h + 1)],
                                   stexp[:, h, :],
                                   start=(i == 0), stop=(i == NT - 1))

                        # normalization, phase-ordered so no DVE op ever
                        # head-of-line-blocks the next pair's PSUM release:
                        # copies free the P@V banks immediately; the
                        # DMA-latency-bound multiplies run last.
                        stages = []
                        for h in range(2):
                            stage = pB.tile([65, 512], f32, name="stage",
                                            tag="stage")
                            nc.vector.tensor_copy(stage[:], pv_ps[h][:])
                            stages.append(stage)
                        dens = []
                        for h in range(2):
                            # [1, 512] DVE reciprocal is FD-bound (~3us); DMA
                            # the denominator row into [128, 4] first where
                            # the same op is ~130ns.
                            den_t = pB.tile([128, 4], f32, name="den_t",
                                            tag="den_t")
                            nc.sync.dma_start(den_t[:], stages[h][64:65, :])
                            dens.append(den_t)
                        rbs = []
                        for h in range(2):
                            nc.vector.reciprocal(dens[h][:], dens[h][:])
                            dr2 = pDr.tile([1, 512], f32, name="dr2", tag="dr2")
                            nc.sync.dma_start(
                                dr2[:].rearrange("p (a b) -> (p a) b", a=128),
                                dens[h][:])
                            # partition-broadcast of the reciprocal row: SBUF
                            # APs can't have zero partition step, so broadcast
                            # from DRAM.
                            rb = pB.tile([64, 512], f32, name="rb", tag="rb")
                            nc.sync.dma_start(rb[:], dr2[:].to_broadcast((64, 512)))
                            rbs.append(rb)
                        for h in range(2):
                            if h == 0:
                                nc.vector.tensor_mul(
                                    aot[0:64, 512 * j:512 * (j + 1)],
                                    stages[0][0:64, :], rbs[0][:])
                            else:
                                tmp = pB.tile([64, 512], fm, name="tmp1",
                                              tag="tmp1")
                                nc.vector.tensor_mul(tmp[:], stages[1][0:64, :],
                                                     rbs[1][:])
                                # DVE lanes cannot shift partitions; DMA moves
                                # the odd head into partitions 64:128.
                                nc.sync.dma_start(
                                    aot[64:128, 512 * j:512 * (j + 1)], tmp[:])

            # ---- output projection: y = attn_out^T.T @ w_proj^T + b ----
            # (opened after phase A closes so wp/bias reuse xt/wqk space)
            with tc.tile_pool(name="proj", bufs=1) as pC:
                wp = [pC.tile([128, C], fm, name=f"wp{k}", tag=f"wp{k}")
                      for k in range(KT)]
                bias_t = pC.tile([128, C], f32, name="bias_t", tag="bias_t")
                for k in range(KT):
                    nc.sync.dma_start(wp[k][:], wprojT_d[128 * k:128 * (k + 1), :])
                nc.sync.dma_start(bias_t[:], bias_d[:])

                for i in range(NT):
                    yt = pB.tile([128, C], f32, name="yt", tag="yt")
                    for c0 in (0, 384):
                        # alternate the two attention psum pools so four
                        # k-accumulation groups can be in flight
                        if (2 * i + c0 // 384) % 2 == 0:
                            pp = psS.tile([128, 384], f32, name="pp", tag="ps")
                        else:
                            pp = psY.tile([128, 384], f32, name="pp", tag="py")
                        for k in range(KT):
                            mm(pp[:, 0:384],
                               aot_all[k][:, 128 * i:128 * (i + 1)],
                               wp[k][:, c0:c0 + 384],
                               start=(k == 0), stop=(k == KT - 1))
                        nc.vector.tensor_add(yt[:, c0:c0 + 384], pp[:, 0:384],
                                             bias_t[:, c0:c0 + 384])
                    nc.sync.dma_start(y_d[128 * i:128 * (i + 1), :], yt[:])

    nc.compile()
    return nc


def round_f32r(a):
    """Round fp32 to the FP32r grid (11 explicit mantissa bits, RNE) --
    what the PE reads for float32r matmuls."""
    a = np.ascontiguousarray(a, dtype=np.float32)
    b = a.view(np.uint32)
    r = (b + np.uint32(0x7FF) + ((b >> np.uint32(12)) & np.uint32(1))) \
        & np.uint32(0xFFFFF000)
    return r.view(np.float32)


def make_in_maps(x, w_qkv, w_proj, b_proj):
    wqkvT = round_f32r(np.asarray(w_qkv, dtype=np.float32).T)
    wprojT = round_f32r(np.asarray(w_proj, dtype=np.float32).T)
    bias_rep = np.ascontiguousarray(
        np.broadcast_to(np.asarray(b_proj, dtype=np.float32), (128, C)))
    x = np.asarray(x, dtype=np.float32)
    return [
        {
            "xT": round_f32r(x[b].T),
            "wqkvT": wqkvT,
            "wprojT": wprojT,
            "bias_rep": bias_rep,
        }
        for b in range(B)
    ]


def kernel(x, w_qkv, w_proj, b_proj):
    from concourse.bass_utils import run_bass_kernel_spmd

    if "nc" not in _CACHE:
        _CACHE["nc"] = build_program()
    nc = _CACHE["nc"]

    in_maps = make_in_maps(x, w_qkv, w_proj, b_proj)
    res = run_bass_kernel_spmd(nc, in_maps, core_ids=list(range(B)))
    out = np.stack([res.results[b]["y"] for b in range(B)], axis=0)
    return out.astype(np.float32)


# revision 42
# speedup vs baseline: 1.1082x; 1.1082x over previous
"""Multi-head attention (B=8, N=1024, C=768, H=12) on 8 Trainium2 NeuronCores.

Sharding: data-parallel, one batch element per core. Each core computes the
full attention block for its batch: QKV projection, per-head softmax(QK^T/8)V,
and the output projection, entirely on-chip (SBUF/PSUM).

Layout strategy (chosen so no on-device transposes are needed):
  - host passes x^T [C, N], w_qkv^T [C, 3C], w_proj^T [C, C], bias replicated
    to [128, C].
  - Q, K are produced transposed ([d, n], head-dim on partitions) by the QKV
    matmul; V is produced in natural [n, d] layout by swapping lhsT/rhs.
  - scores are computed transposed (S^T[m, n] = K Q^T) so that exp(S^T) can be
    consumed directly as the moving operand of the P@V matmul.
  - V tiles carry an appended ones-column, so the P@V matmul's 65th output row
    is the softmax denominator (row-sum of exp scores) for free.
  - normalization multiplies by a reciprocal row broadcast across partitions
    via a DRAM-bounced DMA (SBUF APs cannot partition-broadcast).

Matmul operands use dtype float32r: single-pass PE streaming (1 column/cycle,
4x faster than float32's two-pass LOW/HIGH emulation) with 11 explicit
mantissa bits. Producers round on write; DRAM inputs are pre-rounded on host.

Scheduling: attention for head pair t overlaps the remaining QKV projection
work. All PSUM users run on half-size (single-bank) accumulation groups so
the 8 banks split 2+2 (QK/V projection) + 2+2 (scores / P@V); attn_out^T
tiles reuse the SBUF slots of dead Q^T tiles so everything fits in 192KB.
"""

import sys

import numpy as np

if "/opt/trn_rl_repo" not in sys.path:
    sys.path.insert(0, "/opt/trn_rl_repo")

B = 8
N = 1024
C = 768
H = 12
D = 64
SCALE = D ** -0.5
KT = C // 128           # 6 contraction tiles over channels
MT_QK = 2 * C // 128    # 12 output tiles for Q and K (o in [0, 1536))
NT = N // 128           # 8 token tiles
PAIRS = H // 2          # 6 head pairs

_CACHE = {}


def build_program(fast=True):
    import concourse.bacc as bacc
    import concourse.mybir as mybir
    import concourse.tile as tile

    f32 = mybir.dt.float32
    f32r = mybir.dt.float32r
    Exp = mybir.ActivationFunctionType.Exp
    fm = f32r if fast else f32

    nc = bacc.Bacc("TRN2", target_bir_lowering=False, debug=False)

    xT_d = nc.dram_tensor("xT", [C, N], fm, kind="ExternalInput")
    wqkvT_d = nc.dram_tensor("wqkvT", [C, 3 * C], fm, kind="ExternalInput")
    wprojT_d = nc.dram_tensor("wprojT", [C, C], fm, kind="ExternalInput")
    bias_d = nc.dram_tensor("bias_rep", [128, C], f32, kind="ExternalInput")
    y_d = nc.dram_tensor("y", [N, C], f32, kind="ExternalOutput")

    mm = nc.tensor.matmul

    with tile.TileContext(nc) as tc:
        # qkt/aot share one 12-slot tag: each aot[t] lands in the slot of a
        # Q^T/K^T tile that died right before it (pair t's score matmuls).
        with tc.tile_pool(name="pers", bufs=1) as pers, \
             tc.tile_pool(name="qa", bufs=13) as qa, \
             tc.tile_pool(name="cyc", bufs=2) as pB, \
             tc.tile_pool(name="dramb", bufs=2, space="DRAM") as pDr, \
             tc.tile_pool(name="ps_s", bufs=3, space="PSUM") as psS, \
             tc.tile_pool(name="ps_y", bufs=2, space="PSUM") as psY:
            # Q^T,K^T tiles [d, n]: tile m holds heads 2m (parts 0:64) and
            # 2m+1 (parts 64:128); m 0..5 = Q, 6..11 = K.
            qkt = [qa.tile([128, N], fm, name=f"qkt{m}", tag="qa")
                   for m in range(MT_QK)]
            # V tiles [n-tile, pair, 130]: per pair block [V_h0 |1| V_h1 |1];
            # ones cols at 64 and 129 feed the denominator row of P@V.
            vbuf = [pers.tile([128, PAIRS, 130], fm, name=f"vbuf{i}", tag=f"vbuf{i}")
                    for i in range(NT)]

            with tc.tile_pool(name="phA", bufs=1) as pA:
                xt = [pA.tile([128, N], fm, name=f"xt{k}", tag=f"xt{k}")
                      for k in range(KT)]
                wqk = [pA.tile([128, 2 * C], fm, name=f"wqk{k}", tag=f"wqk{k}")
                       for k in range(KT)]
                wv = [pA.tile([128, C], fm, name=f"wv{k}", tag=f"wv{k}")
                      for k in range(KT)]
                for k in range(KT):
                    nc.sync.dma_start(xt[k][:], xT_d[128 * k:128 * (k + 1), :])
                for k in range(KT):
                    nc.sync.dma_start(wv[k][:],
                                      wqkvT_d[128 * k:128 * (k + 1), 2 * C:3 * C])
                for k in range(KT):
                    nc.sync.dma_start(wqk[k][:],
                                      wqkvT_d[128 * k:128 * (k + 1), 0:2 * C])
                for i in range(NT):
                    ones_ap = vbuf[i].rearrange("p a (t c) -> p a t c", c=65)[:, :, :, 64]
                    nc.vector.memset(ones_ap.bitcast(f32), 1.0)


                # ---- QKV projection, single-bank accumulation groups ----
                def emit_qk(m):
                    for j in range(2):
                        ps = psS.tile([128, 512], f32, name="qk_ps", tag="ps")
                        for k in range(KT):
                            mm(ps[:], wqk[k][:, 128 * m:128 * (m + 1)],
                               xt[k][:, 512 * j:512 * (j + 1)],
                               start=(k == 0), stop=(k == KT - 1))
                        nc.vector.tensor_copy(qkt[m][:, 512 * j:512 * (j + 1)],
                                              ps[:])

                def emit_v(i):
                    for c0, w in ((0, 512), (512, 256)):
                        ps = psY.tile([128, 512], f32, name="v_ps", tag="py")
                        for k in range(KT):
                            mm(ps[:, 0:w], xt[k][:, 128 * i:128 * (i + 1)],
                               wv[k][:, c0:c0 + w],
                               start=(k == 0), stop=(k == KT - 1))
                        # scatter heads: even -> cols 0:64, odd -> cols 65:129
                        # within each 130-wide pair block
                        v_view = ps[:, 0:w].rearrange("p (a t c) -> p a t c",
                                                      t=2, c=64)
                        pa0 = c0 // 128
                        npair = w // 128
                        nc.vector.tensor_copy(
                            vbuf[i][:, pa0:pa0 + npair, 0:64], v_view[:, :, 0, :])
                        nc.vector.tensor_copy(
                            vbuf[i][:, pa0:pa0 + npair, 65:129], v_view[:, :, 1, :])

                # head pairs 0/1 first so attention starts while the rest
                # of the QKV projection still runs; remaining Q/K tiles are
                # emitted interleaved between attention pairs (emission order
                # drives scheduler priority).
                for i in range(NT):
                    emit_v(i)
                for m in (0, 6, 1, 7):
                    emit_qk(m)

                # remaining Q/K half-groups, injected in small chunks inside
                # the attention loops (their own psum pool keeps them off the
                # score-matmul slot chain)
                # ---- attention, j-outer so P@V psum is one bank per head ----
                for t in range(PAIRS):
                    if t + 2 < PAIRS:
                        emit_qk(t + 2)
                        emit_qk(PAIRS + t + 2)
                    qt, kt = qkt[t], qkt[PAIRS + t]
                    aot = qa.tile([128, N], fm, name=f"aot{t}", tag="qa")
                    if t == 0:
                        aot_all = []
                    aot_all.append(aot)
                    for j in range(2):
                        pv_ps = [psY.tile([65, 512], f32, name=f"pv{h}", tag="py")
                                 for h in range(2)]
                        for i in range(NT):
                            stexp = pB.tile([128, 2, 512], fm, name="stexp",
                                            tag="stexp", bufs=4)
                            s_ps = psS.tile([128, 1024], f32, name="s_ps",
                                            tag="ps")
                            for h in range(2):
                                # S^T[m, n] = sum_d K^T[d, m] Q^T[d, n]; h0/h1
                                # use distinct PE row groups (base partition
                                # 0 / 64) and run concurrently.
                                mm(s_ps[:, 512 * h:512 * (h + 1)],
                                   kt[64 * h:64 * (h + 1), 128 * i:128 * (i + 1)],
                                   qt[64 * h:64 * (h + 1), 512 * j:512 * (j + 1)],
                                   start=True, stop=True)
                            # exp(S^T / 8) for both heads, PSUM -> SBUF f32r
                            nc.scalar.activation(
                                stexp[:, :, :],
                                s_ps[:].rearrange("p (h n) -> p h n", h=2),
                                Exp, scale=SCALE)
                            for h in range(2):
                                # rows 0:64 = (P~ @ V)^T, row 64 = denominator
                                mm(pv_ps[h][:],
                                   vbuf[i][:, t, 65 * h:65 * (h + 1)],
                                   stexp[:, h, :],
                                   start=(i == 0), stop=(i == NT - 1))

                        # normalization, phase-ordered so no DVE op ever
                        # head-of-line-blocks the next pair's PSUM release:
                        # copies free the P@V banks immediately; the
                        # DMA-latency-bound multiplies run last.
                        stages = []
                        for h in range(2):
                            stage = pB.tile([65, 512], f32, name="stage",
                                            tag="stage")
                            nc.vector.tensor_copy(stage[:], pv_ps[h][:])
                            stages.append(stage)
                        dens = []
                        for h in range(2):
                            # [1, 512] DVE reciprocal is FD-bound (~3us); DMA
                            # the denominator row into [128, 4] first where
                            # the same op is ~130ns.
                            den_t = pB.tile([128, 4], f32, name="den_t",
                                            tag="den_t")
                            nc.sync.dma_start(den_t[:], stages[h][64:65, :])
                            dens.append(den_t)
                        rbs = []
                        for h in range(2):
                            nc.vector.reciprocal(dens[h][:], dens[h][:])
                            dr2 = pDr.tile([1, 512], f32, name="dr2", tag="dr2")
                            nc.sync.dma_start(
                                dr2[:].rearrange("p (a b) -> (p a) b", a=128),
                                dens[h][:])
                            # partition-broadcast of the reciprocal row: SBUF
                            # APs can't have zero partition step, so broadcast
                            # from DRAM.
                            rb = pB.tile([64, 512], f32, name="rb", tag="rb")
                            nc.sync.dma_start(rb[:], dr2[:].to_broadcast((64, 512)))
                            rbs.append(rb)
                        for h in range(2):
                            if h == 0:
                                nc.vector.tensor_mul(
                                    aot[0:64, 512 * j:512 * (j + 1)],
                                    stages[0][0:64, :], rbs[0][:])
                            else:
                                tmp = pB.tile([64, 512], fm, name="tmp1",
                                              tag="tmp1")
                                nc.vector.tensor_mul(tmp[:], stages[1][0:64, :],
                                                     rbs[1][:])
                                # DVE lanes cannot shift partitions; DMA moves
                                # the odd head into partitions 64:128.
                                nc.sync.dma_start(
                                    aot[64:128, 512 * j:512 * (j + 1)], tmp[:])

            # ---- output projection: y = attn_out^T.T @ w_proj^T + b ----
            # (opened after phase A closes so wp/bias reuse xt/wqk space)
            with tc.tile_pool(name="proj", bufs=1) as pC:
                wp = [pC.tile([128, C], fm, name=f"wp{k}", tag=f"wp{k}")
                      for k in range(KT)]
                bias_t = pC.tile([128, C], f32, name="bias_t", tag="bias_t")
                for k in range(KT):
                    nc.sync.dma_start(wp[k][:], wprojT_d[128 * k:128 * (k + 1), :])
                nc.sync.dma_start(bias_t[:], bias_d[:])

                for i in range(NT):
                    yt = pB.tile([128, C], f32, name="yt", tag="yt")
                    for c0 in (0, 384):
                        # alternate the two attention psum pools so four
                        # k-accumulation groups can be in flight
                        if (2 * i + c0 // 384) % 2 == 0:
                            pp = psS.tile([128, 384], f32, name="pp", tag="ps")
                        else:
                            pp = psY.tile([128, 384], f32, name="pp", tag="py")
                        for k in range(KT):
                            mm(pp[:, 0:384],
                               aot_all[k][:, 128 * i:128 * (i + 1)],
                               wp[k][:, c0:c0 + 384],
                               start=(k == 0), stop=(k == KT - 1))
                        nc.vector.tensor_add(yt[:, c0:c0 + 384], pp[:, 0:384],
                                             bias_t[:, c0:c0 + 384])
                    nc.sync.dma_start(y_d[128 * i:128 * (i + 1), :], yt[:])

    nc.compile()
    return nc


def round_f32r(a):
    """Round fp32 to the FP32r grid (11 explicit mantissa bits, RNE) --
    what the PE reads for float32r matmuls."""
    a = np.ascontiguousarray(a, dtype=np.float32)
    b = a.view(np.uint32)
    r = (b + np.uint32(0x7FF) + ((b >> np.uint32(12)) & np.uint32(1))) \
        & np.uint32(0xFFFFF000)
    return r.view(np.float32)


def make_in_maps(x, w_qkv, w_proj, b_proj):
    wqkvT = round_f32r(np.asarray(w_qkv, dtype=np.float32).T)
    wprojT = round_f32r(np.asarray(w_proj, dtype=np.float32).T)
    bias_rep = np.ascontiguousarray(
        np.broadcast_to(np.asarray(b_proj, dtype=np.float32), (128, C)))
    x = np.asarray(x, dtype=np.float32)
    return [
        {
            "xT": round_f32r(x[b].T),
            "wqkvT": wqkvT,
            "wprojT": wprojT,
            "bias_rep": bias_rep,
        }
        for b in range(B)
    ]


def kernel(x, w_qkv, w_proj, b_proj):
    from concourse.bass_utils import run_bass_kernel_spmd

    if "nc" not in _CACHE:
        _CACHE["nc"] = build_program()
    nc = _CACHE["nc"]

    in_maps = make_in_maps(x, w_qkv, w_proj, b_proj)
    res = run_bass_kernel_spmd(nc, in_maps, core_ids=list(range(B)))
    out = np.stack([res.results[b]["y"] for b in range(B)], axis=0)
    return out.astype(np.float32)
